# revision 4
# baseline (speedup 1.0000x reference)
"""Distributed blocked-cumprod kernel for Trainium2 (8 NeuronCores).

Problem: alpha_bars = cumprod(1 - betas) over T = 2**25 f32 elements.

Strategy (parallel-scan decomposition, pipelined over global chunks):
  - T is split into NCHUNKS global chunks; each chunk is sharded
    contiguously across the 8 cores.  Core k's kernel input is the
    concatenation of its NCHUNKS pieces.
  - Per chunk, per core: view the piece as [128 x cols] row-major.
    ScalarE computes alpha = 1 - beta in place, VectorE runs a chained
    tensor_tensor_scan (op0=mult) -> per-partition local cumprods.
  - Row totals -> PE-transpose -> [1,128] scan -> exclusive prefix;
    core total -> tiny AllGather -> exclusive cross-core prefix
    (selected with a per-core onehot input) x running cross-chunk
    carry; folded into the transpose-back matmul -> full per-partition
    prefix [128,1].
  - ScalarE (VectorE assists on the last chunk) multiplies each tile
    by the prefix in place; DMA out.  Chunk c's exchange + scale +
    store overlap chunk c+1's load + scan.
"""

import sys

sys.path.insert(0, "/opt/trn_rl_repo")

import numpy as np

from concourse import bacc, mybir, tile
from concourse.bass_utils import run_bass_kernel_spmd

NCORES = 8
P = 128
T_FULL = 33554432
TILE_COLS = 2048
NCHUNKS = 2

_F32 = mybir.dt.float32


def build_nc(shard_len: int, tile_cols: int, nchunks: int):
    chunk_len = shard_len // nchunks
    cols = chunk_len // P
    ntiles = cols // tile_cols
    assert chunk_len == P * cols and cols % tile_cols == 0

    mult = mybir.AluOpType.mult
    bypass = mybir.AluOpType.bypass
    add = mybir.AluOpType.add
    identity = mybir.ActivationFunctionType.Identity
    copyf = mybir.ActivationFunctionType.Copy

    nc = bacc.Bacc(num_devices=NCORES)
    betas = nc.declare_dram_parameter("betas", [shard_len], _F32, isOutput=False)
    eye_in = nc.declare_dram_parameter("eye", [P, P], _F32, isOutput=False)
    onehot_in = nc.declare_dram_parameter("onehot", [1, NCORES], _F32, isOutput=False)
    out = nc.declare_dram_parameter("out", [shard_len], _F32, isOutput=True)

    with tile.TileContext(nc) as tc:
        with (
            tc.tile_pool(name="data", bufs=2) as data_pool,
            tc.tile_pool(name="small", bufs=1) as small_pool,
            tc.tile_pool(name="psum", bufs=2, space="PSUM") as psum_pool,
            tc.tile_pool(name="dram", bufs=1, space="DRAM") as dram_pool,
        ):
            eye_sb = small_pool.tile([P, P], _F32, name="eye_sb")
            nc.sync.dma_start(eye_sb[:], eye_in[:, :])
            oh_sb = small_pool.tile([1, NCORES], _F32, name="oh_sb")
            nc.sync.dma_start(oh_sb[:], onehot_in[:, :])

            # Cross-chunk carry (product of all previous chunks' totals).
            carry = [small_pool.tile([1, 1], _F32, name=f"carry{c}") for c in range(nchunks + 1)]
            nc.vector.memset(carry[0][:], 1.0)

            for c in range(nchunks):
                bview = betas.ap()[c * chunk_len : (c + 1) * chunk_len].rearrange(
                    "(p c) -> p c", p=P
                )
                oview = out.ap()[c * chunk_len : (c + 1) * chunk_len].rearrange(
                    "(p c) -> p c", p=P
                )

                A = [
                    data_pool.tile([P, tile_cols], _F32, name=f"a{j}")
                    for j in range(ntiles)
                ]
                # Phase 1: load, alpha = 1 - beta, chained local scan.
                for j in range(ntiles):
                    nc.sync.dma_start(
                        A[j][:], bview[:, j * tile_cols : (j + 1) * tile_cols]
                    )
                for j in range(ntiles):
                    nc.scalar.activation(
                        A[j][:], A[j][:], identity, bias=1.0, scale=-1.0
                    )
                for j in range(ntiles):
                    init = 1.0 if j == 0 else A[j - 1][:, tile_cols - 1 : tile_cols]
                    nc.vector.tensor_tensor_scan(
                        A[j][:], A[j][:], A[j][:], init, mult, bypass
                    )

                # Phase 2: prefixes.
                rowt_ps = psum_pool.tile([1, P], _F32, name="rowt_ps")
                nc.tensor.matmul(
                    rowt_ps[:],
                    A[ntiles - 1][:, tile_cols - 1 : tile_cols],
                    eye_sb[:],
                    start=True,
                    stop=True,
                )
                rowt = small_pool.tile([1, P], _F32, name=f"rowt{c}")
                nc.scalar.copy(rowt[:], rowt_ps[:])
                rowi = small_pool.tile([1, P], _F32, name=f"rowi{c}")
                nc.vector.tensor_tensor_scan(rowi[:], rowt[:], rowt[:], 1.0, mult, bypass)
                exc = small_pool.tile([1, P], _F32, name=f"exc{c}")
                nc.gpsimd.memset(exc[0:1, 0:1], 1.0)
                nc.gpsimd.tensor_copy(exc[0:1, 1:P], rowi[0:1, 0 : P - 1])

                cc_in = dram_pool.tile([1, 1], _F32, name=f"cc_in{c}")
                cc_out = dram_pool.tile(
                    [NCORES, 1], _F32, name=f"cc_out{c}", addr_space="Shared"
                )
                nc.gpsimd.dma_start(cc_in[:], rowi[0:1, P - 1 : P])
                nc.gpsimd.collective_compute(
                    "AllGather",
                    bypass,
                    replica_groups=[list(range(NCORES))],
                    ins=[cc_in.opt()],
                    outs=[cc_out.opt()],
                )
                gath = small_pool.tile([1, NCORES], _F32, name=f"gath{c}")
                nc.gpsimd.dma_start(gath[:], cc_out[:, 0:1])
                gi = small_pool.tile([1, NCORES], _F32, name=f"gi{c}")
                nc.vector.tensor_tensor_scan(gi[:], gath[:], gath[:], 1.0, mult, bypass)
                ge = small_pool.tile([1, NCORES], _F32, name=f"ge{c}")
                nc.gpsimd.memset(ge[0:1, 0:1], 1.0)
                nc.gpsimd.tensor_copy(ge[0:1, 1:NCORES], gi[0:1, 0 : NCORES - 1])
                sel = small_pool.tile([1, NCORES], _F32, name=f"sel{c}")
                nc.vector.tensor_tensor(sel[:], ge[:], oh_sb[:], mult)
                cpref = small_pool.tile([1, 1], _F32, name=f"cpref{c}")
                nc.vector.tensor_reduce(cpref[:], sel[:], mybir.AxisListType.X, add)
                # Fold in the cross-chunk carry; update it for the next chunk.
                sc = small_pool.tile([1, 1], _F32, name=f"sc{c}")
                nc.vector.tensor_tensor(sc[:], cpref[:], carry[c][:], mult)
                if c + 1 < nchunks:
                    nc.vector.tensor_tensor(
                        carry[c + 1][:], carry[c][:], gi[0:1, NCORES - 1 : NCORES], mult
                    )

                fp_ps = psum_pool.tile([P, 1], _F32, name="fp_ps")
                nc.tensor.matmul(fp_ps[:], exc[:], sc[:], start=True, stop=True)
                fprefix = small_pool.tile([P, 1], _F32, name=f"fprefix{c}")
                nc.scalar.copy(fprefix[:], fp_ps[:])

                # Phase 3: scale in place, store.  ScalarE carries the load
                # for all but the last chunk (VectorE is scanning the next
                # chunk); on the last chunk both engines split the tiles.
                for j in range(ntiles):
                    if c == nchunks - 1 and j % 2 == 1:
                        nc.vector.tensor_scalar_mul(A[j][:], A[j][:], fprefix[:])
                    else:
                        nc.scalar.activation(A[j][:], A[j][:], copyf, scale=fprefix[:])
                    nc.sync.dma_start(
                        oview[:, j * tile_cols : (j + 1) * tile_cols], A[j][:]
                    )

    nc.compile()
    return nc


def _shard_slices(total: int):
    """Per-core index ranges: core k owns nchunks pieces."""
    chunk = total // NCHUNKS
    piece = chunk // NCORES
    out = []
    for k in range(NCORES):
        out.append(
            [(c * chunk + k * piece, c * chunk + (k + 1) * piece) for c in range(NCHUNKS)]
        )
    return out


def make_in_maps(betas: np.ndarray):
    eye = np.eye(P, dtype=np.float32)
    slices = _shard_slices(betas.size)
    in_maps = []
    for k in range(NCORES):
        onehot = np.zeros((1, NCORES), dtype=np.float32)
        onehot[0, k] = 1.0
        shard = np.concatenate([betas[a:b] for a, b in slices[k]])
        in_maps.append({"betas": shard, "eye": eye, "onehot": onehot})
    return in_maps


def assemble(results, total: int) -> np.ndarray:
    out = np.empty(total, dtype=np.float32)
    slices = _shard_slices(total)
    piece = total // NCHUNKS // NCORES
    for k in range(NCORES):
        shard = results[k]["out"]
        for c, (a, b) in enumerate(slices[k]):
            out[a:b] = shard[c * piece : (c + 1) * piece]
    return out


def kernel(betas: np.ndarray) -> np.ndarray:
    betas = np.asarray(betas, dtype=np.float32).reshape(-1)
    assert betas.size == T_FULL, betas.size
    nc = build_nc(T_FULL // NCORES, TILE_COLS, NCHUNKS)
    in_maps = make_in_maps(betas)
    res = run_bass_kernel_spmd(nc, in_maps, core_ids=list(range(NCORES)))
    return assemble(res.results, T_FULL)


# revision 5
# speedup vs baseline: 1.6974x; 1.6974x over previous
"""Distributed blocked-cumprod kernel for Trainium2 (8 NeuronCores).

Problem: alpha_bars = cumprod(1 - betas) over T = 2**25 f32 elements.

Strategy (parallel-scan decomposition, pipelined over global chunks):
  - T is split into NCHUNKS global chunks; each chunk is sharded
    contiguously across the 8 cores.  Core k's kernel input is the
    concatenation of its NCHUNKS pieces.
  - Per chunk, per core: view the piece as [128 x cols] row-major.
    ScalarE computes alpha = 1 - beta in place, VectorE runs a chained
    tensor_tensor_scan (op0=mult) -> per-partition local cumprods.
  - Row totals -> PE-transpose -> [1,128] scan -> exclusive prefix;
    core total -> tiny AllGather -> exclusive cross-core prefix
    (selected with a per-core onehot input) x running cross-chunk
    carry; folded into the transpose-back matmul -> full per-partition
    prefix [128,1].
  - ScalarE (VectorE assists on the last chunk) multiplies each tile
    by the prefix in place; DMA out.  Chunk c's exchange + scale +
    store overlap chunk c+1's load + scan.
"""

import sys

sys.path.insert(0, "/opt/trn_rl_repo")

import numpy as np

from concourse import bacc, mybir, tile
from concourse.bass_utils import run_bass_kernel_spmd

NCORES = 8
P = 128
T_FULL = 33554432
TILE_COLS = 2048
NCHUNKS = 2

_F32 = mybir.dt.float32


def build_nc(shard_len: int, tile_cols: int, nchunks: int):
    chunk_len = shard_len // nchunks
    cols = chunk_len // P
    ntiles = cols // tile_cols
    assert chunk_len == P * cols and cols % tile_cols == 0

    mult = mybir.AluOpType.mult
    bypass = mybir.AluOpType.bypass
    add = mybir.AluOpType.add
    identity = mybir.ActivationFunctionType.Identity
    copyf = mybir.ActivationFunctionType.Copy

    nc = bacc.Bacc(num_devices=NCORES)
    betas = nc.declare_dram_parameter("betas", [shard_len], _F32, isOutput=False)
    eye_in = nc.declare_dram_parameter("eye", [P, P], _F32, isOutput=False)
    onehot_in = nc.declare_dram_parameter("onehot", [1, NCORES], _F32, isOutput=False)
    out = nc.declare_dram_parameter("out", [shard_len], _F32, isOutput=True)

    with tile.TileContext(nc) as tc:
        with (
            tc.tile_pool(name="data", bufs=2) as data_pool,
            tc.tile_pool(name="small", bufs=1) as small_pool,
            tc.tile_pool(name="psum", bufs=2, space="PSUM") as psum_pool,
            tc.tile_pool(name="dram", bufs=1, space="DRAM") as dram_pool,
        ):
            eye_sb = small_pool.tile([P, P], _F32, name="eye_sb")
            nc.sync.dma_start(eye_sb[:], eye_in[:, :])
            oh_sb = small_pool.tile([1, NCORES], _F32, name="oh_sb")
            nc.sync.dma_start(oh_sb[:], onehot_in[:, :])

            # Cross-chunk carry (product of all previous chunks' totals).
            carry = [small_pool.tile([1, 1], _F32, name=f"carry{c}") for c in range(nchunks + 1)]
            nc.vector.memset(carry[0][:], 1.0)

            for c in range(nchunks):
                bview = betas.ap()[c * chunk_len : (c + 1) * chunk_len].rearrange(
                    "(p c) -> p c", p=P
                )
                oview = out.ap()[c * chunk_len : (c + 1) * chunk_len].rearrange(
                    "(p c) -> p c", p=P
                )

                A = [
                    data_pool.tile([P, tile_cols], _F32, name=f"a{j}")
                    for j in range(ntiles)
                ]
                # Phase 1: load, alpha = 1 - beta, chained local scan.
                for j in range(ntiles):
                    nc.sync.dma_start(
                        A[j][:], bview[:, j * tile_cols : (j + 1) * tile_cols]
                    )
                for j in range(ntiles):
                    nc.scalar.activation(
                        A[j][:], A[j][:], identity, bias=1.0, scale=-1.0
                    )
                for j in range(ntiles):
                    init = 1.0 if j == 0 else A[j - 1][:, tile_cols - 1 : tile_cols]
                    nc.vector.tensor_tensor_scan(
                        A[j][:], A[j][:], A[j][:], init, mult, bypass
                    )

                # Phase 2: prefixes.
                rowt_ps = psum_pool.tile([1, P], _F32, name="rowt_ps")
                nc.tensor.matmul(
                    rowt_ps[:],
                    A[ntiles - 1][:, tile_cols - 1 : tile_cols],
                    eye_sb[:],
                    start=True,
                    stop=True,
                )
                rowt = small_pool.tile([1, P], _F32, name=f"rowt{c}")
                nc.scalar.copy(rowt[:], rowt_ps[:])
                rowi = small_pool.tile([1, P], _F32, name=f"rowi{c}")
                nc.vector.tensor_tensor_scan(rowi[:], rowt[:], rowt[:], 1.0, mult, bypass)
                exc = small_pool.tile([1, P], _F32, name=f"exc{c}")
                nc.vector.memset(exc[0:1, 0:1], 1.0)
                nc.vector.tensor_copy(exc[0:1, 1:P], rowi[0:1, 0 : P - 1])

                cc_in = dram_pool.tile([1, 1], _F32, name=f"cc_in{c}")
                cc_out = dram_pool.tile(
                    [NCORES, 1], _F32, name=f"cc_out{c}", addr_space="Shared"
                )
                nc.sync.dma_start(cc_in[:], rowi[0:1, P - 1 : P])
                nc.gpsimd.collective_compute(
                    "AllGather",
                    bypass,
                    replica_groups=[list(range(NCORES))],
                    ins=[cc_in.opt()],
                    outs=[cc_out.opt()],
                )
                gath = small_pool.tile([1, NCORES], _F32, name=f"gath{c}")
                nc.sync.dma_start(gath[:], cc_out[:, 0:1])
                gi = small_pool.tile([1, NCORES], _F32, name=f"gi{c}")
                nc.vector.tensor_tensor_scan(gi[:], gath[:], gath[:], 1.0, mult, bypass)
                ge = small_pool.tile([1, NCORES], _F32, name=f"ge{c}")
                nc.vector.memset(ge[0:1, 0:1], 1.0)
                nc.vector.tensor_copy(ge[0:1, 1:NCORES], gi[0:1, 0 : NCORES - 1])
                sel = small_pool.tile([1, NCORES], _F32, name=f"sel{c}")
                nc.vector.tensor_tensor(sel[:], ge[:], oh_sb[:], mult)
                cpref = small_pool.tile([1, 1], _F32, name=f"cpref{c}")
                nc.vector.tensor_reduce(cpref[:], sel[:], mybir.AxisListType.X, add)
                # Fold in the cross-chunk carry; update it for the next chunk.
                sc = small_pool.tile([1, 1], _F32, name=f"sc{c}")
                nc.vector.tensor_tensor(sc[:], cpref[:], carry[c][:], mult)
                if c + 1 < nchunks:
                    nc.vector.tensor_tensor(
                        carry[c + 1][:], carry[c][:], gi[0:1, NCORES - 1 : NCORES], mult
                    )

                fp_ps = psum_pool.tile([P, 1], _F32, name="fp_ps")
                nc.tensor.matmul(fp_ps[:], exc[:], sc[:], start=True, stop=True)
                fprefix = small_pool.tile([P, 1], _F32, name=f"fprefix{c}")
                nc.scalar.copy(fprefix[:], fp_ps[:])

                # Phase 3: scale in place, store.  ScalarE carries the load
                # for all but the last chunk (VectorE is scanning the next
                # chunk); on the last chunk both engines split the tiles.
                for j in range(ntiles):
                    if c == nchunks - 1 and j % 2 == 1:
                        nc.vector.tensor_scalar_mul(A[j][:], A[j][:], fprefix[:])
                    else:
                        nc.scalar.activation(A[j][:], A[j][:], copyf, scale=fprefix[:])
                    nc.sync.dma_start(
                        oview[:, j * tile_cols : (j + 1) * tile_cols], A[j][:]
                    )

    nc.compile()
    return nc


def _shard_slices(total: int):
    """Per-core index ranges: core k owns nchunks pieces."""
    chunk = total // NCHUNKS
    piece = chunk // NCORES
    out = []
    for k in range(NCORES):
        out.append(
            [(c * chunk + k * piece, c * chunk + (k + 1) * piece) for c in range(NCHUNKS)]
        )
    return out


def make_in_maps(betas: np.ndarray):
    eye = np.eye(P, dtype=np.float32)
    slices = _shard_slices(betas.size)
    in_maps = []
    for k in range(NCORES):
        onehot = np.zeros((1, NCORES), dtype=np.float32)
        onehot[0, k] = 1.0
        shard = np.concatenate([betas[a:b] for a, b in slices[k]])
        in_maps.append({"betas": shard, "eye": eye, "onehot": onehot})
    return in_maps


def assemble(results, total: int) -> np.ndarray:
    out = np.empty(total, dtype=np.float32)
    slices = _shard_slices(total)
    piece = total // NCHUNKS // NCORES
    for k in range(NCORES):
        shard = results[k]["out"]
        for c, (a, b) in enumerate(slices[k]):
            out[a:b] = shard[c * piece : (c + 1) * piece]
    return out


def kernel(betas: np.ndarray) -> np.ndarray:
    betas = np.asarray(betas, dtype=np.float32).reshape(-1)
    assert betas.size == T_FULL, betas.size
    nc = build_nc(T_FULL // NCORES, TILE_COLS, NCHUNKS)
    in_maps = make_in_maps(betas)
    res = run_bass_kernel_spmd(nc, in_maps, core_ids=list(range(NCORES)))
    return assemble(res.results, T_FULL)
